# revision 1
# baseline (speedup 1.0000x reference)
"""Hawk (RG-LRU recurrent block) Trainium2 kernel, 8-core SPMD.

Sharding: data-parallel over B (2 groups of 4 cores) x sequence-parallel over T
(4 chunks of 1024 tokens per batch element). The diagonal linear recurrence
h_t = a_t * h_{t-1} + b_t is computed chunk-locally with the hardware
tensor_tensor_scan instruction, then stitched across cores with one small
AllGather of per-chunk scan summaries (A = prod a, b = local final h) and the
per-core correction h = h_local + cumprod(a) * carry.

Layout on device: hidden channels on partitions, time on the free dimension.
All big matmuls run as float32r (full PE rate, ~1.6e-4 rel err).

Per core:
  phase 1: proj (u half) from xT  -> u_pre -> causal depthwise conv -> u_c
  phase 2: ig/rg gate matmuls -> sigmoid -> alpha/xbeta -> scans (h_loc, P)
  collective: AllGather of (A, b) summaries within each batch group
  phase 2.5: proj (gate half) -> gelu  (fills the PE gap during the gather)
  phase 3: carry compose + h correction -> gh = gelu(gate)*h -> out projection
"""
import os

os.environ.setdefault("JAX_COMPILATION_CACHE_DIR", "/tmp/jax_cache_hawk")

import numpy as np

import concourse.bacc as bacc
import concourse.mybir as mybir
import concourse.tile as tile
from concourse.bass_utils import run_bass_kernel_spmd

F32 = mybir.dt.float32
F32R = mybir.dt.float32r
AF = mybir.ActivationFunctionType
OP = mybir.AluOpType

DIM = 1024
HID = 1536
KCONV = 4
B = 2
T = 4096
C_CONST = 8.0

NCORE = 8
TC = T // 4          # tokens per core
NH = HID // 128      # 12 hidden chunks
NDC = DIM // 128     # 8 dim chunks
TPAD = TC + KCONV - 1  # 1027: 3-token left overlap for the causal conv
GBATCH = 3           # gate chunks per ACT-table batch in phase 2

_CACHE: dict = {}


def _build():
    nc = bacc.Bacc("TRN2", target_bir_lowering=False, debug=False,
                   num_devices=NCORE, dynamic_dma_scratch_size=8192)

    xt = nc.dram_tensor("xt", [NDC, 128, TPAD], F32R, kind="ExternalInput").ap()
    wproj = nc.dram_tensor("wproj", [24, NDC, 128, 128], F32R,
                           kind="ExternalInput").ap()
    wgates = nc.dram_tensor("wgates", [2, NH, NH, 128, 128], F32R,
                            kind="ExternalInput").ap()
    wout = nc.dram_tensor("wout", [NDC, NH, 128, 128], F32R,
                          kind="ExternalInput").ap()
    convw = nc.dram_tensor("convw", [NH, 128, 5], F32, kind="ExternalInput").ap()
    gvecs = nc.dram_tensor("gvecs", [NH, 128, 4], F32, kind="ExternalInput").ap()
    sel = nc.dram_tensor("sel", [128, 3], F32, kind="ExternalInput").ap()
    out = nc.dram_tensor("out", [NDC, 128, TC], F32, kind="ExternalOutput").ap()

    with tile.TileContext(nc) as tc:
        with (
            tc.tile_pool(name="big", bufs=47) as big,
            tc.tile_pool(name="wp", bufs=4) as wp,
            tc.tile_pool(name="cst", bufs=1) as cst,
            tc.tile_pool(name="ps", bufs=3, space="PSUM") as ps,
            tc.tile_pool(name="psb", bufs=1, space="PSUM") as psb,
            tc.tile_pool(name="dram", bufs=1, space="DRAM") as dram,
        ):
            # ---- constants ----
            convw_t = []
            gvec_t = []
            for hc in range(NH):
                cw = cst.tile([128, 5], F32, tag=f"cw{hc}", name=f"cw{hc}")
                nc.sync.dma_start(cw[:], convw[hc])
                convw_t.append(cw)
                gv = cst.tile([128, 4], F32, tag=f"gv{hc}", name=f"gv{hc}")
                nc.sync.dma_start(gv[:], gvecs[hc])
                gvec_t.append(gv)
            sel_t = cst.tile([128, 3], F32, tag="sel", name="sel_t")
            nc.sync.dma_start(sel_t[:], sel[:])
            S_loc = cst.tile([128, 24], F32, tag="sloc", name="S_loc")
            G_t = cst.tile([128, 96], F32, tag="gt", name="G_t")
            p2_t = cst.tile([128, NH], F32, tag="p2", name="p2_t")
            p3_t = cst.tile([128, NH], F32, tag="p3", name="p3_t")
            c_t = cst.tile([128, NH], F32, tag="ct", name="c_t")

            # ---- x^T resident ----
            xt_t = []
            for cc in range(NDC):
                x1 = big.tile([128, TPAD], F32R, tag="big", name="x1")
                nc.sync.dma_start(x1[:], xt[cc])
                xt_t.append(x1)

            def proj_matmuls(mc, pt, p3t, wt):
                for th in range(2):
                    for cc in range(NDC):
                        nc.tensor.matmul(
                            pt[:, th * 512:(th + 1) * 512],
                            wt[:, cc * 128:(cc + 1) * 128],
                            xt_t[cc][:, 3 + th * 512: 3 + (th + 1) * 512],
                            start=(cc == 0), stop=(cc == NDC - 1))
                if p3t is not None:
                    # f32r matmuls need a multiple-of-4 free size: compute 4
                    # prefix columns, only the first 3 are used.
                    for cc in range(NDC):
                        nc.tensor.matmul(
                            p3t[:, 0:4],
                            wt[:, cc * 128:(cc + 1) * 128],
                            xt_t[cc][:, 0:4],
                            start=(cc == 0), stop=(cc == NDC - 1))

            def load_wproj(mc):
                wt = wp.tile([128, NDC * 128], F32R, tag="w", name="wt")
                nc.sync.dma_start(
                    wt[:].rearrange("k (c m) -> k c m", c=NDC),
                    wproj[mc].rearrange("c k m -> k c m"))
                return wt

            # ---- phase 1: u half of proj + causal conv ----
            u_c = []
            for mc in range(NH):
                wt = load_wproj(mc)
                pt = ps.tile([128, TC], F32, tag="ps", name="pt")
                p3t = psb.tile([128, TC], F32, tag="psb", name="p3t")
                proj_matmuls(mc, pt, p3t, wt)
                upre = big.tile([128, TPAD], F32, tag="big", name="upre")
                nc.scalar.copy(upre[:, 3:TPAD], pt[:])
                nc.vector.tensor_copy(upre[:, 0:3], p3t[:, 0:3])
                uc = big.tile([128, TPAD], F32R, tag="big", name="uc")
                w5 = convw_t[mc]
                # u_c[t] = sum_k w_k * u_pre[t-3+k] + conv_b
                nc.vector.tensor_scalar(
                    uc[:, 0:TC], upre[:, 0:TC], w5[:, 0:1], w5[:, 4:5],
                    OP.mult, OP.add)
                for k in range(1, KCONV):
                    nc.vector.scalar_tensor_tensor(
                        uc[:, 0:TC], upre[:, k:k + TC], w5[:, k:k + 1],
                        uc[:, 0:TC].bitcast(F32), OP.mult, OP.add)
                u_c.append(uc)

            # ---- phase 2: gates + scans, in ACT-table batches ----
            h_loc: list = [None] * NH
            P_dram = []
            for gc in range(NH):
                P_dram.append(dram.tile([128, TC], F32, tag=f"pd{gc}", name=f"pd{gc}"))

            def emit_2a(gcs, A_t, X_t):
                for gc in gcs:
                    gv = gvec_t[gc]
                    pig = ps.tile([128, TC], F32, tag="ps", name="pig")
                    prg = ps.tile([128, TC], F32, tag="ps", name="prg")
                    for dst, wi in ((pig, 0), (prg, 1)):
                        wt = wp.tile([128, NH * 128], F32R, tag="w", name="wt")
                        nc.sync.dma_start(
                            wt[:].rearrange("k (h m) -> k h m", h=NH),
                            wgates[wi, gc].rearrange("h k m -> k h m"))
                        for th in range(2):
                            for hc in range(NH):
                                nc.tensor.matmul(
                                    dst[:, th * 512:(th + 1) * 512],
                                    wt[:, hc * 128:(hc + 1) * 128],
                                    u_c[hc][:, th * 512:(th + 1) * 512],
                                    start=(hc == 0), stop=(hc == NH - 1))
                    at = big.tile([128, TPAD], F32, tag="big", name="at")
                    nc.scalar.activation(at[:, 0:TC], prg[:], AF.Sigmoid,
                                         bias=gv[:, 2:3])
                    xt_ = big.tile([128, TPAD], F32, tag="big", name="xt_")
                    nc.scalar.activation(xt_[:, 0:TC], pig[:], AF.Sigmoid,
                                         bias=gv[:, 1:2])
                    nc.vector.tensor_tensor(
                        xt_[:, 0:TC], xt_[:, 0:TC],
                        u_c[gc][:, 0:TC].bitcast(F32), OP.mult)
                    A_t[gc] = at
                    X_t[gc] = xt_

            def emit_2b(gcs, A_t, X_t):
                B2_t: dict = {}
                for gc in gcs:
                    gv = gvec_t[gc]
                    # A_t holds sigmoid(rg); alpha = exp(-rate*sig),
                    # alpha^2 = exp(-2*rate*sig) via per-partition scales
                    b2 = big.tile([128, TPAD], F32, tag="big", name="b2")
                    nc.scalar.activation(b2[:, 0:TC], A_t[gc][:, 0:TC],
                                         AF.Exp, scale=gv[:, 3:4])
                    nc.scalar.activation(A_t[gc][:, 0:TC], A_t[gc][:, 0:TC],
                                         AF.Exp, scale=gv[:, 0:1])
                    B2_t[gc] = b2
                for gc in gcs:
                    at, xt_, b2 = A_t[gc], X_t[gc], B2_t[gc]
                    nc.vector.tensor_scalar(b2[:, 0:TC], b2[:, 0:TC],
                                            -1.0, 1.000001, OP.mult, OP.add)
                    nc.scalar.activation(b2[:, 0:TC], b2[:, 0:TC], AF.Sqrt)
                    nc.vector.tensor_tensor(xt_[:, 0:TC], xt_[:, 0:TC],
                                            b2[:, 0:TC], OP.mult)
                    hl = big.tile([128, TPAD], F32, tag="big", name="hl")
                    nc.vector.tensor_tensor_scan(
                        hl[:, 0:TC], at[:, 0:TC], xt_[:, 0:TC], 0.0,
                        OP.mult, OP.add)
                    pt_ = big.tile([128, TPAD], F32, tag="big", name="pt_")
                    nc.vector.tensor_tensor_scan(
                        pt_[:, 0:TC], at[:, 0:TC], at[:, 0:TC], 1.0,
                        OP.mult, OP.bypass)
                    nc.vector.tensor_copy(S_loc[:, gc:gc + 1],
                                          pt_[:, TC - 1:TC])
                    nc.vector.tensor_copy(S_loc[:, 12 + gc:13 + gc],
                                          hl[:, TC - 1:TC])
                    nc.sync.dma_start(P_dram[gc][:], pt_[:, 0:TC])
                    h_loc[gc] = hl

            # Tapered batches: the scan chain of the last batch drains
            # after the final gate matmul with the PE idle, so keep the
            # tail batches small (earlier chains overlap later matmuls).
            sizes = [3, 3, 3, 2, 1]
            starts = [sum(sizes[:i]) for i in range(len(sizes))]
            batches = [range(s0, s0 + sz) for s0, sz in zip(starts, sizes)]
            pend = None
            for gcs in batches:
                A_t: dict = {}
                X_t: dict = {}
                emit_2a(gcs, A_t, X_t)
                if pend is not None:
                    emit_2b(*pend)
                pend = (gcs, A_t, X_t)
            emit_2b(*pend)

            # ---- phase 2.5: gate half of proj + gelu (fills the gather gap) --
            gg_t = []
            for i in range(NH):
                wt = load_wproj(NH + i)
                pt = psb.tile([128, TC], F32, tag="psb", name="pt")
                proj_matmuls(NH + i, pt, None, wt)
                gg = big.tile([128, TPAD], F32, tag="big", name="gg")
                nc.scalar.activation(gg[:, 0:TC], pt[:], AF.Gelu)
                gg_t.append(gg)

            # ---- collective: gather (A, b) summaries within batch group ----
            cin = dram.tile([128, 24], F32, tag="cin", name="cin")
            cout = dram.tile([4, 128, 24], F32, tag="cout", name="cout")
            nc.sync.dma_start(cin[:], S_loc[:])
            nc.gpsimd.collective_compute(
                "AllGather", OP.bypass,
                replica_groups=[[0, 1, 2, 3], [4, 5, 6, 7]],
                ins=[cin.opt()], outs=[cout.opt()])
            for r in range(4):
                nc.sync.dma_start(G_t[:, r * 24:(r + 1) * 24], cout[r])

            # ---- carry composition ----
            # G columns: r*24 + [0:12]=A_r, [12:24]=b_r.  p1 = b_0
            # p2 = A_1*p1 + b_1 ; p3 = A_2*p2 + b_2 ; c = sum_r sel_r * p_r
            p1 = G_t[:, 12:24]
            nc.vector.tensor_tensor(p2_t[:], G_t[:, 24:36], p1, OP.mult)
            nc.vector.tensor_tensor(p2_t[:], p2_t[:], G_t[:, 36:48], OP.add)
            nc.vector.tensor_tensor(p3_t[:], G_t[:, 48:60], p2_t[:], OP.mult)
            nc.vector.tensor_tensor(p3_t[:], p3_t[:], G_t[:, 60:72], OP.add)
            nc.vector.tensor_scalar(c_t[:], p1, sel_t[:, 0:1], None, OP.mult)
            nc.vector.scalar_tensor_tensor(c_t[:], p2_t[:], sel_t[:, 1:2],
                                           c_t[:], OP.mult, OP.add)
            nc.vector.scalar_tensor_tensor(c_t[:], p3_t[:], sel_t[:, 2:3],
                                           c_t[:], OP.mult, OP.add)

            # ---- phase 3: correction, gh, output projection ----
            gh_t = []
            for gc in range(NH):
                pin = big.tile([128, TPAD], F32, tag="big", name="pin")
                nc.sync.dma_start(pin[:, 0:TC], P_dram[gc][:])
                hl = h_loc[gc]
                nc.vector.scalar_tensor_tensor(
                    hl[:, 0:TC], pin[:, 0:TC], c_t[:, gc:gc + 1],
                    hl[:, 0:TC], OP.mult, OP.add)
                gh = big.tile([128, TPAD], F32R, tag="big", name="gh")
                nc.vector.tensor_tensor(gh[:, 0:TC], gg_t[gc][:, 0:TC],
                                        hl[:, 0:TC], OP.mult)
                gh_t.append(gh)

            for dc in range(NDC):
                wt = wp.tile([128, NH * 128], F32R, tag="w", name="wt")
                nc.sync.dma_start(
                    wt[:].rearrange("k (h m) -> k h m", h=NH),
                    wout[dc].rearrange("h k m -> k h m"))
                po = ps.tile([128, TC], F32, tag="ps", name="po")
                for th in range(2):
                    for gc in range(NH):
                        nc.tensor.matmul(
                            po[:, th * 512:(th + 1) * 512],
                            wt[:, gc * 128:(gc + 1) * 128],
                            gh_t[gc][:, th * 512:(th + 1) * 512],
                            start=(gc == 0), stop=(gc == NH - 1))
                ot = big.tile([128, TPAD], F32, tag="big", name="ot")
                nc.scalar.copy(ot[:, 0:TC], po[:])
                nc.sync.dma_start(out[dc], ot[:, 0:TC])

    nc.compile()
    return nc


def _softplus64(x):
    x = np.asarray(x, np.float64)
    return np.log1p(np.exp(-np.abs(x))) + np.maximum(x, 0.0)


def _prepare(x, W_proj, conv_w, conv_b, W_in, b_in, W_gate, b_gate,
             forget_lambda, W_out):
    x = np.asarray(x, np.float32)
    W_proj = np.asarray(W_proj, np.float32)
    conv_w = np.asarray(conv_w, np.float32)
    conv_b = np.asarray(conv_b, np.float32)
    W_in = np.asarray(W_in, np.float32)
    b_in = np.asarray(b_in, np.float32)
    W_gate = np.asarray(W_gate, np.float32)
    b_gate = np.asarray(b_gate, np.float32)
    forget_lambda = np.asarray(forget_lambda, np.float32)
    W_out = np.asarray(W_out, np.float32)

    # wproj blocks: [mc, cc, k, m]; mc 0..11 = u rows (1536:3072),
    # mc 12..23 = gate rows (0:1536)
    wp_ = W_proj.reshape(24, 128, NDC, 128).transpose(0, 2, 3, 1)
    order = list(range(12, 24)) + list(range(0, 12))
    wproj = np.ascontiguousarray(wp_[order])

    win_ = W_in.reshape(NH, 128, NH, 128).transpose(0, 2, 3, 1)
    wgt_ = W_gate.reshape(NH, 128, NH, 128).transpose(0, 2, 3, 1)
    wgates = np.ascontiguousarray(np.stack([win_, wgt_]))

    wout = np.ascontiguousarray(
        W_out.reshape(NDC, 128, NH, 128).transpose(0, 2, 3, 1))

    convw = np.concatenate(
        [conv_w[:, 0, :].reshape(NH, 128, KCONV),
         conv_b.reshape(NH, 128, 1)], axis=2).astype(np.float32)
    convw = np.ascontiguousarray(convw)

    negrate = (-C_CONST * _softplus64(forget_lambda)).astype(np.float32)
    gvecs = np.ascontiguousarray(np.stack(
        [negrate.reshape(NH, 128),
         b_in.reshape(NH, 128),
         b_gate.reshape(NH, 128),
         2.0 * negrate.reshape(NH, 128)], axis=2))

    in_maps = []
    for c in range(NCORE):
        bb, j = divmod(c, 4)
        lo = j * TC - (KCONV - 1)
        if lo < 0:
            chunk = np.concatenate(
                [np.zeros((KCONV - 1, DIM), np.float32), x[bb, 0:(j + 1) * TC]])
        else:
            chunk = x[bb, lo:(j + 1) * TC]
        xtc = np.ascontiguousarray(chunk.T).reshape(NDC, 128, TPAD)
        selc = np.zeros((128, 3), np.float32)
        if j > 0:
            selc[:, j - 1] = 1.0
        in_maps.append({
            "xt": xtc, "wproj": wproj, "wgates": wgates, "wout": wout,
            "convw": convw, "gvecs": gvecs, "sel": selc,
        })
    return in_maps


def _get_nc():
    if "nc" not in _CACHE:
        _CACHE["nc"] = _build()
    return _CACHE["nc"]


def kernel(x, W_proj, conv_w, conv_b, W_in, b_in, W_gate, b_gate,
           forget_lambda, W_out):
    nc = _get_nc()
    in_maps = _prepare(x, W_proj, conv_w, conv_b, W_in, b_in, W_gate, b_gate,
                       forget_lambda, W_out)
    res = run_bass_kernel_spmd(nc, in_maps, core_ids=list(range(NCORE)))
    out = np.empty((B, T, DIM), np.float32)
    for c in range(NCORE):
        bb, j = divmod(c, 4)
        o = res.results[c]["out"].reshape(DIM, TC)
        out[bb, j * TC:(j + 1) * TC, :] = o.T
    return out



# revision 27
# speedup vs baseline: 1.1168x; 1.1168x over previous
"""Hawk (RG-LRU recurrent block) Trainium2 kernel, 8-core SPMD.

Sharding: data-parallel over B (2 groups of 4 cores) x sequence-parallel over T
(4 chunks of 1024 tokens per batch element). The diagonal linear recurrence
h_t = a_t * h_{t-1} + b_t is computed chunk-locally with the hardware
tensor_tensor_scan instruction, then stitched across cores with one small
AllGather of per-chunk scan summaries (A = prod a, b = local final h) and the
per-core correction h = h_local + cumprod(a) * carry.

Layout on device: hidden channels on partitions, time on the free dimension.
All big matmuls run as float32r (full PE rate, ~1.6e-4 rel err).

Per core:
  phase 1: proj (u half) from xT  -> u_pre -> causal depthwise conv -> u_c
  phase 2: ig/rg gate matmuls -> sigmoid -> alpha/xbeta -> scans (h_loc, P)
  collective: AllGather of (A, b) summaries within each batch group
  phase 2.5: proj (gate half) -> gelu  (fills the PE gap during the gather)
  phase 3: carry compose + h correction -> gh = gelu(gate)*h -> out projection
"""
import os

os.environ.setdefault("JAX_COMPILATION_CACHE_DIR", "/tmp/jax_cache_hawk")

import numpy as np

import concourse.bacc as bacc
import concourse.mybir as mybir
import concourse.tile as tile
from concourse.bass_utils import run_bass_kernel_spmd

F32 = mybir.dt.float32
F32R = mybir.dt.float32r
AF = mybir.ActivationFunctionType
OP = mybir.AluOpType

DIM = 1024
HID = 1536
KCONV = 4
B = 2
T = 4096
C_CONST = 8.0

NCORE = 8
TC = T // 4          # tokens per core
NH = HID // 128      # 12 hidden chunks
NDC = DIM // 128     # 8 dim chunks
TPAD = TC + KCONV - 1  # 1027: 3-token left overlap for the causal conv
GBATCH = 3           # gate chunks per ACT-table batch in phase 2

_CACHE: dict = {}


def _build():
    nc = bacc.Bacc("TRN2", target_bir_lowering=False, debug=False,
                   num_devices=NCORE, dynamic_dma_scratch_size=8192)

    xt = nc.dram_tensor("xt", [NDC, 128, TPAD], F32R, kind="ExternalInput").ap()
    wproj = nc.dram_tensor("wproj", [24, NDC, 128, 128], F32R,
                           kind="ExternalInput").ap()
    wgates = nc.dram_tensor("wgates", [2, NH, NH, 128, 128], F32R,
                            kind="ExternalInput").ap()
    wout = nc.dram_tensor("wout", [NDC, NH, 128, 128], F32R,
                          kind="ExternalInput").ap()
    convw = nc.dram_tensor("convw", [NH, 128, 5], F32, kind="ExternalInput").ap()
    gvecs = nc.dram_tensor("gvecs", [NH, 128, 4], F32, kind="ExternalInput").ap()
    sel = nc.dram_tensor("sel", [128, 3], F32, kind="ExternalInput").ap()
    out = nc.dram_tensor("out", [NDC, 128, TC], F32, kind="ExternalOutput").ap()

    with tile.TileContext(nc) as tc:
        with (
            tc.tile_pool(name="big", bufs=47) as big,
            tc.tile_pool(name="wp", bufs=4) as wp,
            tc.tile_pool(name="cst", bufs=1) as cst,
            tc.tile_pool(name="ps", bufs=3, space="PSUM") as ps,
            tc.tile_pool(name="psb", bufs=1, space="PSUM") as psb,
            tc.tile_pool(name="dram", bufs=1, space="DRAM") as dram,
        ):
            # ---- constants ----
            convw_t = []
            gvec_t = []
            for hc in range(NH):
                cw = cst.tile([128, 5], F32, tag=f"cw{hc}", name=f"cw{hc}")
                nc.sync.dma_start(cw[:], convw[hc])
                convw_t.append(cw)
                gv = cst.tile([128, 4], F32, tag=f"gv{hc}", name=f"gv{hc}")
                nc.sync.dma_start(gv[:], gvecs[hc])
                gvec_t.append(gv)
            sel_t = cst.tile([128, 3], F32, tag="sel", name="sel_t")
            nc.sync.dma_start(sel_t[:], sel[:])
            S_loc = cst.tile([128, 24], F32, tag="sloc", name="S_loc")
            G_t = cst.tile([128, 96], F32, tag="gt", name="G_t")
            p2_t = cst.tile([128, NH], F32, tag="p2", name="p2_t")
            p3_t = cst.tile([128, NH], F32, tag="p3", name="p3_t")
            c_t = cst.tile([128, NH], F32, tag="ct", name="c_t")

            # ---- x^T resident ----
            xt_t = []
            for cc in range(NDC):
                x1 = big.tile([128, TPAD], F32R, tag="big", name="x1")
                nc.sync.dma_start(x1[:], xt[cc])
                xt_t.append(x1)

            def proj_matmuls(mc, pt, p3t, wt):
                for th in range(2):
                    for cc in range(NDC):
                        nc.tensor.matmul(
                            pt[:, th * 512:(th + 1) * 512],
                            wt[:, cc * 128:(cc + 1) * 128],
                            xt_t[cc][:, 3 + th * 512: 3 + (th + 1) * 512],
                            start=(cc == 0), stop=(cc == NDC - 1))
                if p3t is not None:
                    # f32r matmuls need a multiple-of-4 free size: compute 4
                    # prefix columns, only the first 3 are used.
                    for cc in range(NDC):
                        nc.tensor.matmul(
                            p3t[:, 0:4],
                            wt[:, cc * 128:(cc + 1) * 128],
                            xt_t[cc][:, 0:4],
                            start=(cc == 0), stop=(cc == NDC - 1))

            def load_wproj(mc):
                wt = wp.tile([128, NDC * 128], F32R, tag="w", name="wt")
                nc.sync.dma_start(
                    wt[:].rearrange("k (c m) -> k c m", c=NDC),
                    wproj[mc].rearrange("c k m -> k c m"))
                return wt

            # ---- phase 1: u half of proj + causal conv ----
            u_c = []
            for mc in range(NH):
                wt = load_wproj(mc)
                pt = ps.tile([128, TC], F32, tag="ps", name="pt")
                p3t = psb.tile([128, TC], F32, tag="psb", name="p3t")
                proj_matmuls(mc, pt, p3t, wt)
                upre = big.tile([128, TPAD], F32, tag="big", name="upre")
                nc.scalar.copy(upre[:, 3:TPAD], pt[:])
                nc.vector.tensor_copy(upre[:, 0:3], p3t[:, 0:3])
                uc = big.tile([128, TPAD], F32R, tag="big", name="uc")
                w5 = convw_t[mc]
                # u_c[t] = sum_k w_k * u_pre[t-3+k] + conv_b
                nc.vector.tensor_scalar(
                    uc[:, 0:TC], upre[:, 0:TC], w5[:, 0:1], w5[:, 4:5],
                    OP.mult, OP.add)
                for k in range(1, KCONV):
                    nc.vector.scalar_tensor_tensor(
                        uc[:, 0:TC], upre[:, k:k + TC], w5[:, k:k + 1],
                        uc[:, 0:TC].bitcast(F32), OP.mult, OP.add)
                u_c.append(uc)

            # ---- phase 2: gates + scans, in ACT-table batches ----
            h_loc: list = [None] * NH
            P_dram = []
            for gc in range(NH):
                P_dram.append(dram.tile([128, TC], F32, tag=f"pd{gc}", name=f"pd{gc}"))

            def emit_2a(gcs, A_t, X_t):
                for gc in gcs:
                    gv = gvec_t[gc]
                    pig = ps.tile([128, TC], F32, tag="ps", name="pig")
                    prg = ps.tile([128, TC], F32, tag="ps", name="prg")
                    for dst, wi in ((pig, 0), (prg, 1)):
                        wt = wp.tile([128, NH * 128], F32R, tag="w", name="wt")
                        nc.sync.dma_start(
                            wt[:].rearrange("k (h m) -> k h m", h=NH),
                            wgates[wi, gc].rearrange("h k m -> k h m"))
                        for th in range(2):
                            for hc in range(NH):
                                nc.tensor.matmul(
                                    dst[:, th * 512:(th + 1) * 512],
                                    wt[:, hc * 128:(hc + 1) * 128],
                                    u_c[hc][:, th * 512:(th + 1) * 512],
                                    start=(hc == 0), stop=(hc == NH - 1))
                    at = big.tile([128, TPAD], F32, tag="big", name="at")
                    nc.scalar.activation(at[:, 0:TC], prg[:], AF.Sigmoid,
                                         bias=gv[:, 2:3])
                    xt_ = big.tile([128, TPAD], F32, tag="big", name="xt_")
                    nc.scalar.activation(xt_[:, 0:TC], pig[:], AF.Sigmoid,
                                         bias=gv[:, 1:2])
                    nc.vector.tensor_tensor(
                        xt_[:, 0:TC], xt_[:, 0:TC],
                        u_c[gc][:, 0:TC].bitcast(F32), OP.mult)
                    A_t[gc] = at
                    X_t[gc] = xt_

            def emit_2b(gcs, A_t, X_t):
                B2_t: dict = {}
                for gc in gcs:
                    gv = gvec_t[gc]
                    # A_t holds sigmoid(rg); alpha = exp(-rate*sig),
                    # alpha^2 = exp(-2*rate*sig) via per-partition scales
                    b2 = big.tile([128, TPAD], F32, tag="big", name="b2")
                    nc.scalar.activation(b2[:, 0:TC], A_t[gc][:, 0:TC],
                                         AF.Exp, scale=gv[:, 3:4])
                    nc.scalar.activation(A_t[gc][:, 0:TC], A_t[gc][:, 0:TC],
                                         AF.Exp, scale=gv[:, 0:1])
                    B2_t[gc] = b2
                for gc in gcs:
                    at, xt_, b2 = A_t[gc], X_t[gc], B2_t[gc]
                    nc.vector.tensor_scalar(b2[:, 0:TC], b2[:, 0:TC],
                                            -1.0, 1.000001, OP.mult, OP.add)
                    nc.scalar.activation(b2[:, 0:TC], b2[:, 0:TC], AF.Sqrt)
                    nc.vector.tensor_tensor(xt_[:, 0:TC], xt_[:, 0:TC],
                                            b2[:, 0:TC], OP.mult)
                    hl = big.tile([128, TPAD], F32, tag="big", name="hl")
                    nc.vector.tensor_tensor_scan(
                        hl[:, 0:TC], at[:, 0:TC], xt_[:, 0:TC], 0.0,
                        OP.mult, OP.add)
                    pt_ = big.tile([128, TPAD], F32, tag="big", name="pt_")
                    nc.vector.tensor_tensor_scan(
                        pt_[:, 0:TC], at[:, 0:TC], at[:, 0:TC], 1.0,
                        OP.mult, OP.bypass)
                    nc.vector.tensor_copy(S_loc[:, gc:gc + 1],
                                          pt_[:, TC - 1:TC])
                    nc.vector.tensor_copy(S_loc[:, 12 + gc:13 + gc],
                                          hl[:, TC - 1:TC])
                    nc.sync.dma_start(P_dram[gc][:], pt_[:, 0:TC])
                    h_loc[gc] = hl

            # Tapered batches: the scan chain of the last batch drains
            # after the final gate matmul with the PE idle, so keep the
            # tail batches small (earlier chains overlap later matmuls).
            sizes = [3, 3, 3, 2, 1]
            starts = [sum(sizes[:i]) for i in range(len(sizes))]
            batches = [range(s0, s0 + sz) for s0, sz in zip(starts, sizes)]
            pend = None
            for gcs in batches:
                A_t: dict = {}
                X_t: dict = {}
                emit_2a(gcs, A_t, X_t)
                if pend is not None:
                    emit_2b(*pend)
                pend = (gcs, A_t, X_t)
            emit_2b(*pend)

            # ---- phase 2.5: gate half of proj + gelu (fills the gather gap) --
            gg_t = []
            for i in range(NH):
                wt = load_wproj(NH + i)
                pt = psb.tile([128, TC], F32, tag="psb", name="pt")
                proj_matmuls(NH + i, pt, None, wt)
                gg = big.tile([128, TPAD], F32, tag="big", name="gg")
                nc.scalar.activation(gg[:, 0:TC], pt[:], AF.Gelu)
                gg_t.append(gg)

            # ---- collective: gather (A, b) summaries within batch group ----
            cin = dram.tile([128, 24], F32, tag="cin", name="cin")
            cout = dram.tile([4, 128, 24], F32, tag="cout", name="cout")
            nc.sync.dma_start(cin[:], S_loc[:])
            nc.gpsimd.collective_compute(
                "AllGather", OP.bypass,
                replica_groups=[[0, 1, 2, 3], [4, 5, 6, 7]],
                ins=[cin.opt()], outs=[cout.opt()])
            for r in range(4):
                nc.sync.dma_start(G_t[:, r * 24:(r + 1) * 24], cout[r])

            # ---- carry composition ----
            # G columns: r*24 + [0:12]=A_r, [12:24]=b_r.  p1 = b_0
            # p2 = A_1*p1 + b_1 ; p3 = A_2*p2 + b_2 ; c = sum_r sel_r * p_r
            p1 = G_t[:, 12:24]
            nc.vector.tensor_tensor(p2_t[:], G_t[:, 24:36], p1, OP.mult)
            nc.vector.tensor_tensor(p2_t[:], p2_t[:], G_t[:, 36:48], OP.add)
            nc.vector.tensor_tensor(p3_t[:], G_t[:, 48:60], p2_t[:], OP.mult)
            nc.vector.tensor_tensor(p3_t[:], p3_t[:], G_t[:, 60:72], OP.add)
            nc.vector.tensor_scalar(c_t[:], p1, sel_t[:, 0:1], None, OP.mult)
            nc.vector.scalar_tensor_tensor(c_t[:], p2_t[:], sel_t[:, 1:2],
                                           c_t[:], OP.mult, OP.add)
            nc.vector.scalar_tensor_tensor(c_t[:], p3_t[:], sel_t[:, 2:3],
                                           c_t[:], OP.mult, OP.add)

            # ---- phase 3: correction, gh, output projection ----
            gh_t = []
            for gc in range(NH):
                pin = big.tile([128, TPAD], F32, tag="big", name="pin")
                nc.sync.dma_start(pin[:, 0:TC], P_dram[gc][:])
                hl = h_loc[gc]
                nc.vector.scalar_tensor_tensor(
                    hl[:, 0:TC], pin[:, 0:TC], c_t[:, gc:gc + 1],
                    hl[:, 0:TC], OP.mult, OP.add)
                gh = big.tile([128, TPAD], F32R, tag="big", name="gh")
                nc.vector.tensor_tensor(gh[:, 0:TC], gg_t[gc][:, 0:TC],
                                        hl[:, 0:TC], OP.mult)
                gh_t.append(gh)

            for dc in range(NDC):
                wt = wp.tile([128, NH * 128], F32R, tag="w", name="wt")
                nc.sync.dma_start(
                    wt[:].rearrange("k (h m) -> k h m", h=NH),
                    wout[dc].rearrange("h k m -> k h m"))
                po = ps.tile([128, TC], F32, tag="ps", name="po")
                for th in range(2):
                    for gc in range(NH):
                        nc.tensor.matmul(
                            po[:, th * 512:(th + 1) * 512],
                            wt[:, gc * 128:(gc + 1) * 128],
                            gh_t[gc][:, th * 512:(th + 1) * 512],
                            start=(gc == 0), stop=(gc == NH - 1))
                ot = big.tile([128, TPAD], F32, tag="big", name="ot")
                nc.scalar.copy(ot[:, 0:TC], po[:])
                nc.sync.dma_start(out[dc], ot[:, 0:TC])

    nc.compile()
    return nc


def _softplus64(x):
    x = np.asarray(x, np.float64)
    return np.log1p(np.exp(-np.abs(x))) + np.maximum(x, 0.0)


def _prepare(x, W_proj, conv_w, conv_b, W_in, b_in, W_gate, b_gate,
             forget_lambda, W_out):
    x = np.asarray(x, np.float32)
    W_proj = np.asarray(W_proj, np.float32)
    conv_w = np.asarray(conv_w, np.float32)
    conv_b = np.asarray(conv_b, np.float32)
    W_in = np.asarray(W_in, np.float32)
    b_in = np.asarray(b_in, np.float32)
    W_gate = np.asarray(W_gate, np.float32)
    b_gate = np.asarray(b_gate, np.float32)
    forget_lambda = np.asarray(forget_lambda, np.float32)
    W_out = np.asarray(W_out, np.float32)

    # wproj blocks: [mc, cc, k, m]; mc 0..11 = u rows (1536:3072),
    # mc 12..23 = gate rows (0:1536)
    wp_ = W_proj.reshape(24, 128, NDC, 128).transpose(0, 2, 3, 1)
    order = list(range(12, 24)) + list(range(0, 12))
    wproj = np.ascontiguousarray(wp_[order])

    win_ = W_in.reshape(NH, 128, NH, 128).transpose(0, 2, 3, 1)
    wgt_ = W_gate.reshape(NH, 128, NH, 128).transpose(0, 2, 3, 1)
    wgates = np.ascontiguousarray(np.stack([win_, wgt_]))

    wout = np.ascontiguousarray(
        W_out.reshape(NDC, 128, NH, 128).transpose(0, 2, 3, 1))

    convw = np.concatenate(
        [conv_w[:, 0, :].reshape(NH, 128, KCONV),
         conv_b.reshape(NH, 128, 1)], axis=2).astype(np.float32)
    convw = np.ascontiguousarray(convw)

    negrate = (-C_CONST * _softplus64(forget_lambda)).astype(np.float32)
    gvecs = np.ascontiguousarray(np.stack(
        [negrate.reshape(NH, 128),
         b_in.reshape(NH, 128),
         b_gate.reshape(NH, 128),
         2.0 * negrate.reshape(NH, 128)], axis=2))

    in_maps = []
    for c in range(NCORE):
        bb, j = divmod(c, 4)
        lo = j * TC - (KCONV - 1)
        if lo < 0:
            chunk = np.concatenate(
                [np.zeros((KCONV - 1, DIM), np.float32), x[bb, 0:(j + 1) * TC]])
        else:
            chunk = x[bb, lo:(j + 1) * TC]
        xtc = np.ascontiguousarray(chunk.T).reshape(NDC, 128, TPAD)
        selc = np.zeros((128, 3), np.float32)
        if j > 0:
            selc[:, j - 1] = 1.0
        in_maps.append({
            "xt": xtc, "wproj": wproj, "wgates": wgates, "wout": wout,
            "convw": convw, "gvecs": gvecs, "sel": selc,
        })
    return in_maps


def _get_nc():
    if "nc" not in _CACHE:
        _CACHE["nc"] = _build()
    return _CACHE["nc"]


def kernel(x, W_proj, conv_w, conv_b, W_in, b_in, W_gate, b_gate,
           forget_lambda, W_out):
    nc = _get_nc()
    in_maps = _prepare(x, W_proj, conv_w, conv_b, W_in, b_in, W_gate, b_gate,
                       forget_lambda, W_out)
    res = run_bass_kernel_spmd(nc, in_maps, core_ids=list(range(NCORE)))
    out = np.empty((B, T, DIM), np.float32)
    for c in range(NCORE):
        bb, j = divmod(c, 4)
        o = res.results[c]["out"].reshape(DIM, TC)
        out[bb, j * TC:(j + 1) * TC, :] = o.T
    return out

